# revision 26
# baseline (speedup 1.0000x reference)
"""GQA attention (B=1, S=2048, H=2048, 32 q-heads / 8 kv-heads, hd=64)
on 8 Trainium2 NeuronCores.

Sharding: tensor-parallel over heads. Core c owns q-heads 4c..4c+3 and
kv-head c. Each core AllToAlls its normalized attention output (bf16,
256 KB per q-chunk collective) so core r ends up with attnT
[2048 head-dims, seq] for its own interleaved 64-row sequence stripes,
then computes o_proj locally against the FULL wo (bf16, SBUF-resident,
prefetched during attention). Core r owns output rows
{512*qc + 64*r + [0,64)}.

All matmul inputs are bf16 (1 cyc/row on the PE at any width, half the
DMA/SBUF traffic); PSUM accumulation stays fp32 and the RMSNorm / RoPE
/ softmax-stat math stays fp32/f32r on DVE/ScalarE.

Schedule (program order = engine queue order per engine):
  qh0-A proj -> qh0-B norm/rope -> qh1-A proj   (qh1-A matmuls hide
    qh0-B's DVE chain)
  CDE qc0 -> A2A0 -> qc1 -> A2A1 -> qh1-B (DVE overlaps qc0/1 TensorE)
  qc2 -> o_proj rows{qc0,qc1} -> qc3 -> A2A3 -> o_proj rows{qc2,qc3}
PSUM banks: psA(6) + psO(2) during projections; psS(4) + psPV(2) +
psO(2) during attention (psA closed first).
"""
import sys

sys.path.insert(0, "/opt/trn_rl_repo")

import numpy as np  # noqa: E402
import concourse.bacc as bacc  # noqa: E402
import concourse.mybir as mybir  # noqa: E402
import concourse.tile as tile  # noqa: E402
from concourse import bass_utils  # noqa: E402

f32 = mybir.dt.float32
f32r = mybir.dt.float32r
bf16 = mybir.dt.bfloat16
AF = mybir.ActivationFunctionType
BF16_NP = mybir.dt.np(bf16)

N_CORES = 8
S = 2048
HID = 2048
HD = 64
ROPE_THETA = 10000.0
RMS_EPS = 1e-6
SCALING = HD ** -0.5              # 0.125
NK = HID // 128                   # 16 contraction tiles
NQC = S // 512                    # 4 q chunks
NKT = S // 128                    # 16 kpos tiles

_NC_CACHE = None
LAST_RESULTS = None


def _build():
    nc = bacc.Bacc("TRN2", target_bir_lowering=False, debug=False,
                   num_devices=N_CORES)

    def din(name, shape, dt):
        return nc.dram_tensor(name, shape, dt, kind="ExternalInput").ap()

    xT = din("xT", [HID, S], bf16)
    # host-pretiled: row p, col block t = original rows 128t+p
    wq0 = din("wq0", [128, HID], bf16)
    wq1 = din("wq1", [128, HID], bf16)
    wkv = din("wkv", [128, HID], bf16)     # [wv | wk] columns pretiled
    wof = din("wof", [128, NK * HID], bf16)  # full wo, ktile-blocked
    cos2 = din("cos2", [128, S], f32)
    ss2 = din("ss2", [128, S], f32)
    ew_q = din("ew_q", [2, 128], f32r)
    ew_k = din("ew_k", [2, 128], f32r)
    e2 = din("e2", [2, 128], f32r)
    e2t = din("e2t", [128, 2], f32r)
    mask = din("mask", [128, 128], bf16)
    ident = din("ident", [64, 64], f32)

    out_hw = nc.dram_tensor("out", [256, S], f32,
                            kind="ExternalOutput").ap()

    with tile.TileContext(nc) as tc:
        with tc.tile_pool(name="consts", bufs=1) as cp, \
             tc.tile_pool(name="dram", bufs=1, space="DRAM") as dp, \
             tc.tile_pool(name="sbB", bufs=2) as sbB, \
             tc.tile_pool(name="sbC", bufs=4) as sbC, \
             tc.tile_pool(name="psO", bufs=2, space="PSUM") as psO:
            c_wq0 = cp.tile([128, HID], bf16, tag="w")
            c_wq1 = cp.tile([128, HID], bf16, tag="w2")
            c_wkv = cp.tile([128, HID], bf16, tag="w3")
            c_wof = cp.tile([128, NK * HID], bf16, tag="w4")
            c_cos = cp.tile([128, S], f32, tag="c1")
            c_ss = cp.tile([128, S], f32, tag="c2")
            c_ewq = cp.tile([2, 128], f32r, tag="c3")
            c_ewk = cp.tile([2, 128], f32r, tag="c4")
            c_e2 = cp.tile([2, 128], f32r, tag="c5")
            c_e2t = cp.tile([128, 2], f32r, tag="c5t")
            c_mask = cp.tile([128, 128], bf16, tag="c6")
            c_id = cp.tile([64, 64], f32, tag="c7")
            c_eps = cp.tile([2, 1], f32, tag="c8")

            # phase-A weights first (contiguous rows);
            # wq0 complete first so the first matmul unblocks early
            nc.sync.dma_start(c_wq0[:], wq0)
            nc.scalar.dma_start(c_wq1[:], wq1)
            nc.scalar.dma_start(c_wkv[:], wkv)
            nc.vector.memset(c_eps[:], RMS_EPS)
            nc.scalar.dma_start(c_e2t[:], e2t)
            nc.scalar.dma_start(c_ewq[:], ew_q)
            nc.scalar.dma_start(c_ewk[:], ew_k)
            nc.scalar.dma_start(c_id[:], ident)
            nc.scalar.dma_start(c_cos[:], cos2)
            nc.scalar.dma_start(c_ss[:], ss2)

            qr0 = cp.tile([128, S], bf16, tag="qr0")
            qr1 = cp.tile([128, S], bf16, tag="qr1")
            krd = cp.tile([128, S], bf16, tag="krd")
            v_aug = cp.tile([128, NKT * (HD + 1)], bf16, tag="vaug")

            a2a_in = [dp.tile([2048, 64], bf16, tag=f"ain{q}",
                              name=f"a2a_in{q}") for q in range(3)]
            a2a_out = [dp.tile([2048, 64], bf16, tag=f"aout{q}",
                               name=f"a2a_out{q}") for q in range(3)]
            a2a_in3 = [dp.tile([1024, 64], bf16, tag=f"ain3{h}",
                               name=f"a2a_in3{h}") for h in range(2)]
            a2a_out3 = [dp.tile([1024, 64], bf16, tag=f"aout3{h}",
                                name=f"a2a_out3{h}") for h in range(2)]

            specs = [
                ("kv", c_ewk, krd, True),
                ("q0", c_ewq, qr0, False),
                ("q1", c_ewq, qr1, False),
            ]
            qkv = {}   # per-half projection outputs, f32 [128, 1024]

            pqs = {}
            sqs = {}

            def qkv_copy(qh, j):
                key = ("q0", "q1", "kv")[j]
                dst = sbB.tile([128, 1024], f32, tag=f"qkv{j}",
                               bufs=1, name=f"qkv{qh}_{key}")
                nc.vector.tensor_copy(dst[:], pqs[qh][j][:])
                qkv[(qh, key)] = dst

            def stat_sq(qh, si):
                key = specs[si][0]
                sq = sbB.tile([128, 1024], f32r, tag="sq",
                              bufs=2, name=f"sq{qh}_{si}")
                nc.vector.tensor_mul(sq[:], qkv[(qh, key)][:],
                                     qkv[(qh, key)][:])
                sqs[(qh, si)] = sq

            def phase_a(qh, xp, psA, copies=True):
                """Projections for 1024-col half qh -> qkv f32 tiles."""
                hs = slice(1024 * qh, 1024 * qh + 1024)
                pq = [psA.tile([128, 1024], f32, tag="pa",
                               name=f"pa{qh}_{j}") for j in range(3)]
                for t in range(NK):
                    xt = xp.tile([128, 1024], bf16, tag="xt")
                    nc.sync.dma_start(xt[:],
                                      xT[128 * t:128 * (t + 1), hs])
                    st = (t == 0)
                    sp = (t == NK - 1)
                    tc_ = slice(128 * t, 128 * (t + 1))
                    for j, w in ((0, c_wq0), (1, c_wq1), (2, c_wkv)):
                        nc.tensor.matmul(pq[j][:, 0:512], w[:, tc_],
                                         xt[:, 0:512],
                                         start=st, stop=sp)
                        nc.tensor.matmul(pq[j][:, 512:1024], w[:, tc_],
                                         xt[:, 512:1024],
                                         start=st, stop=sp)
                pqs[qh] = pq
                if copies:
                    for j in range(3):
                        qkv_copy(qh, j)

            def phase_b_stats(qh):
                """RMS variance stats for half qh: partition sums via
                ones-block matmul, rstd = Exp(-0.5 Ln(var)) in place
                (batched per table: 2 ACT table loads)."""
                rstds = {}
                lnvs = {}
                for si in range(3):
                    if (qh, si) not in sqs:
                        stat_sq(qh, si)
                    sq = sqs[(qh, si)]
                    for u in range(2):
                        us = slice(512 * u, 512 * u + 512)
                        pss = psO.tile([2, 512], f32, tag="o",
                                       name=f"ss{qh}_{si}_{u}")
                        nc.tensor.matmul(pss[:], c_e2t[:], sq[:, us],
                                         start=True, stop=True)
                        lnv = sbB.tile([2, 512], f32r, tag="lnv",
                                       bufs=6, name=f"lnv{qh}{si}{u}")
                        nc.scalar.activation(lnv[:], pss[:], AF.Ln,
                                             scale=1.0 / HD,
                                             bias=c_eps[:])
                        lnvs[(si, u)] = lnv
                for si in range(3):
                    for u in range(2):
                        lnv = lnvs[(si, u)]
                        nc.scalar.activation(lnv[:], lnv[:],
                                             AF.Exp, scale=-0.5)
                        rstds[(si, u)] = lnv
                return rstds

            def phase_b_norm(qh, u, rstds):
                """Norm + RoPE for 512-col quarter (qh, u): DVE/DMA/
                TensorE only (no ScalarE), plus V transposes."""
                cs = slice(1024 * qh + 512 * u, 1024 * qh + 512 * u + 512)
                us = slice(512 * u, 512 * u + 512)
                for si, (key, ew, dst, is_kv) in enumerate(specs):
                    src = qkv[(qh, key)]
                    rows = slice(64, 128) if is_kv else slice(0, 128)
                    nrm = sbB.tile([128, 512], f32, tag="nrm",
                                   bufs=2, name=f"nrm{qh}{u}_{si}")
                    pb = psO.tile([128, 512], f32, tag="o",
                                  name=f"pb{qh}_{si}_{u}")
                    nc.tensor.matmul(pb[:], ew[:], rstds[(si, u)][:],
                                     start=True, stop=True)
                    nc.vector.tensor_mul(nrm[rows, :], src[rows, us],
                                         pb[rows, :])
                    # rope
                    sh = sbB.tile([128, 512], f32, tag="sh",
                                  bufs=2, name=f"sh{qh}{u}_{si}")
                    if is_kv:
                        nc.sync.dma_start(sh[64:96, :], nrm[96:128, :])
                        nc.sync.dma_start(sh[96:128, :], nrm[64:96, :])
                    else:
                        nc.sync.dma_start(sh[0:32, :], nrm[32:64, :])
                        nc.sync.dma_start(sh[32:64, :], nrm[0:32, :])
                        nc.sync.dma_start(sh[64:96, :], nrm[96:128, :])
                        nc.sync.dma_start(sh[96:128, :], nrm[64:96, :])
                    t2 = sbB.tile([128, 512], f32, tag="sq",
                                  bufs=2, name=f"t2{qh}{u}_{si}")
                    nc.vector.tensor_mul(t2[rows, :], sh[rows, :],
                                         c_ss[rows, cs])
                    t1 = sbB.tile([128, 512], f32, tag="sh",
                                  bufs=2, name=f"t1{qh}{u}_{si}")
                    nc.vector.tensor_mul(t1[rows, :], nrm[rows, :],
                                         c_cos[rows, cs])
                    nc.vector.tensor_add(dst[rows, cs], t1[rows, :],
                                         t2[rows, :])
                    if is_kv:
                        nc.sync.dma_start(dst[0:64, cs],
                                          dst[64:128, cs])
                        if qh == 0 and u == 0:
                            nc.gpsimd.memset(v_aug[:], 1.0)
                        for tt in range(8 * qh + 4 * u,
                                        8 * qh + 4 * u + 4):
                            lc = 128 * tt - 1024 * qh
                            ptr = psO.tile([128, 64], f32, tag="o",
                                           name=f"pt{qh}_{tt}")
                            nc.tensor.transpose(
                                ptr[:], src[0:64, lc:lc + 128], c_id[:])
                            nc.vector.tensor_copy(
                                v_aug[:,
                                      (HD + 1) * tt:(HD + 1) * tt + HD],
                                ptr[:])

            def cde_chunk(qc, psS, psPV, mid=None):
                """Scores + PV + normalize + A2A staging for q-chunk."""
                for hp, qr in ((0, qr0), (1, qr1)):
                    if hp == 1 and mid is not None:
                        mid()
                    ppv_a = psPV.tile([65, 512], f32, tag="pv")
                    ppv_b = psPV.tile([65, 512], f32, tag="pv")
                    ntile = 4 * qc + 4
                    for t in range(ntile):
                        r = t - 4 * qc
                        off = max(0, r) * 128
                        qlo = 512 * qc + off
                        qlen = 512 * (qc + 1) - qlo
                        kc = slice(128 * t, 128 * (t + 1))
                        vs = slice((HD + 1) * t, (HD + 1) * t + HD + 1)
                        st = (t == 0)
                        sp = (t == ntile - 1)
                        ps_s = psS.tile([128, 1024], f32, tag="s")
                        nc.tensor.matmul(
                            ps_s[:, 0:qlen], krd[0:64, kc],
                            qr[0:64, qlo:qlo + qlen],
                            start=True, stop=True)
                        nc.tensor.matmul(
                            ps_s[:, 512:512 + qlen], krd[64:128, kc],
                            qr[64:128, qlo:qlo + qlen],
                            start=True, stop=True)
                        pt = sbC.tile([128, 1024], bf16, tag="pt", bufs=3)
                        if r >= 0:
                            nc.scalar.activation(
                                pt[:, 0:qlen], ps_s[:, 0:qlen],
                                AF.Exp, scale=SCALING)
                            nc.scalar.activation(
                                pt[:, 512:512 + qlen],
                                ps_s[:, 512:512 + qlen],
                                AF.Exp, scale=SCALING)
                            nc.vector.tensor_mul(
                                pt[:, 0:128], pt[:, 0:128], c_mask[:])
                            nc.vector.tensor_mul(
                                pt[:, 512:640], pt[:, 512:640],
                                c_mask[:])
                        else:
                            nc.scalar.activation(
                                pt[:, 0:1024], ps_s[:, 0:1024],
                                AF.Exp, scale=SCALING)
                        nc.tensor.matmul(
                            ppv_a[:, off:512], v_aug[:, vs],
                            pt[:, 0:qlen], start=st, stop=sp)
                        nc.tensor.matmul(
                            ppv_b[:, off:512], v_aug[:, vs],
                            pt[:, 512:512 + qlen], start=st, stop=sp)
                    araw = sbC.tile([128, 512], f32, tag="araw",
                                    bufs=2, name=f"araw{qc}_{hp}")
                    lsb = sbC.tile([2, 512], f32, tag="lsb", bufs=1,
                                   name=f"lsb{qc}_{hp}")
                    for half, ppv in ((0, ppv_a), (1, ppv_b)):
                        stg = sbC.tile([65, 512], f32, tag="stg",
                                       bufs=2)
                        nc.vector.tensor_copy(stg[:], ppv[:])
                        nc.sync.dma_start(
                            araw[64 * half:64 * half + 64, :],
                            stg[0:64, :])
                        nc.sync.dma_start(
                            lsb[half:half + 1, :], stg[64:65, :])
                    # normalize: 1/l on DVE (keeps ScalarE exp table
                    # resident), broadcast via ones-block matmul
                    rl = sbC.tile([2, 512], f32, tag="lnl", bufs=1,
                                  name=f"rl{qc}_{hp}")
                    nc.vector.reciprocal_approx_fast(rl[:], lsb[:])
                    rl_r = sbC.tile([2, 512], f32r, tag="rlr", bufs=1,
                                    name=f"rlr{qc}_{hp}")
                    nc.vector.tensor_copy(rl_r[:], rl[:])
                    pb = psO.tile([128, 512], f32, tag="o")
                    nc.tensor.matmul(pb[:], c_e2[:], rl_r[:],
                                     start=True, stop=True)
                    abf = sbC.tile([128, 512], bf16, tag="abf",
                                   bufs=2, name=f"abf{qc}_{hp}")
                    nc.vector.tensor_mul(abf[:], araw[:], pb[:])
                    # stage bf16 attn into the AllToAll input:
                    # owner r gets seq cols 64r..64r+64 of this chunk.
                    # qc3 exchanges per-hp so hp0's collective overlaps
                    # hp1's attention (shortens the tail).
                    abf_r = abf[:].rearrange("p (r c) -> p r c",
                                             r=8, c=64)
                    if qc == 3:
                        nc.scalar.dma_start(
                            a2a_in3[hp][:].rearrange(
                                "(r p) c -> p r c", r=8, p=128),
                            abf_r)
                        nc.gpsimd.collective_compute(
                            "AllToAll",
                            mybir.AluOpType.bypass,
                            replica_groups=[list(range(N_CORES))],
                            ins=[a2a_in3[hp][:].opt()],
                            outs=[a2a_out3[hp][:].opt()],
                        )
                    else:
                        nc.scalar.dma_start(
                            a2a_in[qc][:].rearrange(
                                "(r h p) c -> h p r c",
                                r=8, h=2, p=128)[hp],
                            abf_r)
                if qc != 3:
                    nc.gpsimd.collective_compute(
                        "AllToAll",
                        mybir.AluOpType.bypass,
                        replica_groups=[list(range(N_CORES))],
                        ins=[a2a_in[qc][:].opt()],
                        outs=[a2a_out[qc][:].opt()],
                    )

            psSp = [None]
            lhs_t = [sbC.tile([128, NK * 128], bf16, tag="lhs",
                              bufs=2, name=f"lhsA{p}") for p in range(2)]

            def lhs_load(p, part):
                """part 0: chunk2p + (p1: hp0-even) cols; part 1: odd"""
                lhs = lhs_t[p]
                lhs2 = lhs[:].rearrange("p (t b c) -> b p t c",
                                        t=16, b=2, c=64)
                if part == 0:
                    nc.gpsimd.dma_start(
                        lhs2[0],
                        a2a_out[2 * p][:].rearrange(
                            "(t p) c -> p t c", t=16, p=128))
                    if p == 0:
                        nc.gpsimd.dma_start(
                            lhs2[1],
                            a2a_out[1][:].rearrange(
                                "(t p) c -> p t c", t=16, p=128))
                elif part == 2:
                    lhs4 = lhs[:].rearrange(
                        "p (j q b c) -> q b p j c",
                        j=8, q=2, b=2, c=64)
                    nc.gpsimd.dma_start(
                        lhs4[0][1],
                        a2a_out3[0][:].rearrange(
                            "(t p) c -> p t c", t=8, p=128))
                else:
                    lhs4 = lhs_t[1][:].rearrange(
                        "p (j q b c) -> q b p j c", j=8, q=2, b=2, c=64)
                    nc.gpsimd.dma_start(
                        lhs4[1][1],
                        a2a_out3[1][:].rearrange(
                            "(t p) c -> p t c", t=8, p=128))

            def oproj_rowpair(p):
                """o_proj for owner rows from q-chunks 2p, 2p+1."""
                lhs = lhs_t[p]
                if p == 1:
                    lhs_load(1, 1)
                ost = sbC.tile([128, S], f32, tag="ost", bufs=1,
                               name=f"ost{p}")

                def mm(po, n, t, st, sp):
                    nc.tensor.matmul(
                        po[:], lhs[:, 128 * t:128 * (t + 1)],
                        c_wof[:, HID * t + 512 * n:
                              HID * t + 512 * n + 512],
                        start=st, stop=sp)

                if p == 1:
                    # three waves: chunk2-only (needs just A2A_2),
                    # chunk3-even (hp0 exchange), chunk3-odd (hp1) —
                    # waves 1-2 fill the collective-wait gaps. Extra
                    # accumulators come from the now-idle psS pool.
                    def mmh(po, rows, n, t, cols, st, sp):
                        nc.tensor.matmul(
                            po[rows, :], lhs[:, cols],
                            c_wof[:, HID * t + 512 * n:
                                  HID * t + 512 * n + 512],
                            start=st, stop=sp)

                    pos = []
                    r2 = slice(0, 64)
                    r3 = slice(64, 128)
                    for n in range(4):
                        pool = psO if n < 2 else psSp[0]
                        po = pool.tile([128, 512], f32,
                                       tag="o" if n < 2 else "s",
                                       name=f"poe{n}")
                        pos.append(po)
                        for t in range(NK):   # wave 1: chunk2 cols
                            mmh(po, r2, n, t,
                                slice(128 * t, 128 * t + 64),
                                t == 0, t == NK - 1)
                    for n in range(4):        # wave 2: chunk3 even t
                        po = pos[n]
                        for i, t in enumerate(range(0, NK, 2)):
                            mmh(po, r3, n, t,
                                slice(128 * t + 64, 128 * t + 128),
                                i == 0, False)
                    for n in range(4):        # wave 3: chunk3 odd t
                        po = pos[n]
                        for i, t in enumerate(range(1, NK, 2)):
                            mmh(po, r3, n, t,
                                slice(128 * t + 64, 128 * t + 128),
                                False, i == NK // 2 - 1)
                        nc.vector.tensor_copy(
                            ost[:, 512 * n:512 * n + 512], po[:])
                        nc.sync.dma_start(
                            out_hw[128 * p:128 * (p + 1),
                                   512 * n:512 * n + 512],
                            ost[:, 512 * n:512 * n + 512])
                else:
                    for n in range(4):
                        po = psO.tile([128, 512], f32, tag="o")
                        for t in range(NK):
                            mm(po, n, t, t == 0, t == NK - 1)
                        nc.vector.tensor_copy(
                            ost[:, 512 * n:512 * n + 512], po[:])
                    nc.sync.dma_start(out_hw[128 * p:128 * (p + 1), :],
                                      ost[:])

            # ---- Projections: qh0 A+B, then qh1 A (hides qh0-B) ----
            with tc.tile_pool(name="xt", bufs=4) as xp, \
                 tc.tile_pool(name="psA", bufs=3, space="PSUM") as psA:
                phase_a(0, xp, psA)
                r0 = phase_b_stats(0)
                phase_b_norm(0, 0, r0)
                phase_b_norm(0, 1, r0)
                phase_a(1, xp, psA)

            # consts for CDE + full-wo prefetch (streams during qc0-2)
            nc.scalar.dma_start(c_e2[:], e2)
            nc.scalar.dma_start(c_mask[:], mask)
            for t in range(NK):
                ws = slice(HID * t, HID * (t + 1))
                nc.sync.dma_start(c_wof[:, ws], wof[:, ws])

            # ---- Attention + o_proj ----
            with tc.tile_pool(name="psS", bufs=2, space="PSUM") as psS, \
                 tc.tile_pool(name="psPV", bufs=2, space="PSUM") as psPV:
                psSp[0] = psS
                stat_sq(1, 0)            # spec si=0 -> kv
                cde_chunk(0, psS, psPV,
                          mid=lambda: stat_sq(1, 1))
                stat_sq(1, 2)            # spec si=2 -> q1
                r1h = []

                def mid1():
                    r1h.append(phase_b_stats(1))
                    phase_b_norm(1, 0, r1h[0])

                cde_chunk(1, psS, psPV, mid=mid1)
                phase_b_norm(1, 1, r1h[0])  # DVE overlaps qc2 TensorE
                cde_chunk(2, psS, psPV,
                          mid=lambda: lhs_load(0, 0))
                oproj_rowpair(0)
                lhs_load(1, 0)           # chunk2 cols: after A2A_2
                cde_chunk(3, psS, psPV,
                          mid=lambda: lhs_load(1, 2))
                oproj_rowpair(1)

    nc.compile()
    return nc


def _host_prep(hidden_states, position_ids, wq, wk, wv, wo, q_ln_w, k_ln_w):
    x = np.asarray(hidden_states, dtype=np.float32)[0]        # [S, HID]
    xT = np.ascontiguousarray(x.T).astype(BF16_NP)            # [HID, S]
    pos = np.asarray(position_ids)[0].astype(np.float32)      # [S]
    inv = 1.0 / (ROPE_THETA ** (np.arange(0, HD, 2, dtype=np.float32) / HD))
    ang = pos[:, None] * inv[None, :]                         # [S, 32]
    emb = np.concatenate([ang, ang], axis=1)                  # [S, 64]
    cosT = np.cos(emb).T.astype(np.float32)                   # [64, S]
    sinT = np.sin(emb).T.astype(np.float32)
    ss = sinT.copy()
    ss[0:32] = -sinT[0:32]
    cos2 = np.tile(cosT, (2, 1))
    ss2 = np.tile(ss, (2, 1))

    e2 = np.zeros((2, 128), dtype=np.float32)
    e2[0, 0:64] = 1.0
    e2[1, 64:128] = 1.0
    ew_q = np.zeros((2, 128), dtype=np.float32)
    ew_q[0, 0:64] = q_ln_w
    ew_q[1, 64:128] = q_ln_w
    ew_k = np.zeros((2, 128), dtype=np.float32)
    ew_k[1, 64:128] = k_ln_w
    msk = (np.arange(128)[:, None] <= np.arange(128)[None, :]) \
        .astype(BF16_NP)
    ident = np.eye(64, dtype=np.float32)

    wq_ = np.asarray(wq, dtype=np.float32)
    wk_ = np.asarray(wk, dtype=np.float32)
    wv_ = np.asarray(wv, dtype=np.float32)
    wo_ = np.asarray(wo, dtype=np.float32)

    def pretile(w):  # [HID, n] -> [128, NK*n] ktile-blocked
        n = w.shape[1]
        return np.ascontiguousarray(
            w.reshape(NK, 128, n).transpose(1, 0, 2).reshape(128, NK * n)
        ).astype(BF16_NP)

    wof = pretile(wo_)                                        # [128, 32768]

    in_maps = []
    for c in range(N_CORES):
        qcols = slice(256 * c, 256 * (c + 1))
        kvcols = slice(64 * c, 64 * (c + 1))
        wq_c = np.ascontiguousarray(wq_[:, qcols])
        wkv_c = np.concatenate([wv_[:, kvcols], wk_[:, kvcols]], axis=1)
        in_maps.append({
            "xT": xT,
            "wq0": pretile(wq_c[:, 0:128]),
            "wq1": pretile(wq_c[:, 128:256]),
            "wkv": pretile(wkv_c),
            "wof": wof,
            "cos2": cos2,
            "ss2": ss2,
            "ew_q": ew_q,
            "ew_k": ew_k,
            "e2": e2,
            "e2t": np.ascontiguousarray(e2.T),
            "mask": msk,
            "ident": ident,
        })
    return in_maps


def kernel(hidden_states, position_ids, wq, wk, wv, wo, q_ln_w, k_ln_w):
    global _NC_CACHE, LAST_RESULTS
    if _NC_CACHE is None:
        _NC_CACHE = _build()
    nc = _NC_CACHE
    in_maps = _host_prep(hidden_states, position_ids, wq, wk, wv, wo,
                         q_ln_w, k_ln_w)
    res = bass_utils.run_bass_kernel_spmd(
        nc, in_maps, core_ids=list(range(N_CORES)))
    LAST_RESULTS = res
    out = np.empty((S, HID), dtype=np.float32)
    for c in range(N_CORES):
        o_c = res.results[c]["out"]           # [256, 2048]
        for qc in range(NQC):
            out[512 * qc + 64 * c:512 * qc + 64 * c + 64, :] = \
                o_c[64 * qc:64 * qc + 64, :]
    return out.reshape(1, S, HID)


# revision 27
# speedup vs baseline: 1.1068x; 1.1068x over previous
"""GQA attention (B=1, S=2048, H=2048, 32 q-heads / 8 kv-heads, hd=64)
on 8 Trainium2 NeuronCores.

Sharding: tensor-parallel over heads. Core c owns q-heads 4c..4c+3 and
kv-head c. Each core AllToAlls its normalized attention output (bf16,
256 KB per q-chunk collective) so core r ends up with attnT
[2048 head-dims, seq] for its own interleaved 64-row sequence stripes,
then computes o_proj locally against the FULL wo (bf16, SBUF-resident,
prefetched during attention). Core r owns output rows
{512*qc + 64*r + [0,64)}.

All matmul inputs are bf16 (1 cyc/row on the PE at any width, half the
DMA/SBUF traffic); PSUM accumulation stays fp32 and the RMSNorm / RoPE
/ softmax-stat math stays fp32/f32r on DVE/ScalarE.

Schedule (program order = engine queue order per engine):
  qh0-A proj -> qh0-B norm/rope -> qh1-A proj   (qh1-A matmuls hide
    qh0-B's DVE chain)
  CDE qc0 -> A2A0 -> qc1 -> A2A1 -> qh1-B (DVE overlaps qc0/1 TensorE)
  qc2 -> o_proj rows{qc0,qc1} -> qc3 -> A2A3 -> o_proj rows{qc2,qc3}
PSUM banks: psA(6) + psO(2) during projections; psS(4) + psPV(2) +
psO(2) during attention (psA closed first).
"""
import sys

sys.path.insert(0, "/opt/trn_rl_repo")

import numpy as np  # noqa: E402
import concourse.bacc as bacc  # noqa: E402
import concourse.mybir as mybir  # noqa: E402
import concourse.tile as tile  # noqa: E402
from concourse import bass_utils  # noqa: E402

f32 = mybir.dt.float32
f32r = mybir.dt.float32r
bf16 = mybir.dt.bfloat16
AF = mybir.ActivationFunctionType
BF16_NP = mybir.dt.np(bf16)

N_CORES = 8
S = 2048
HID = 2048
HD = 64
ROPE_THETA = 10000.0
RMS_EPS = 1e-6
SCALING = HD ** -0.5              # 0.125
NK = HID // 128                   # 16 contraction tiles
NQC = S // 512                    # 4 q chunks
NKT = S // 128                    # 16 kpos tiles

_NC_CACHE = None
LAST_RESULTS = None


def _build():
    nc = bacc.Bacc("TRN2", target_bir_lowering=False, debug=False,
                   num_devices=N_CORES)

    def din(name, shape, dt):
        return nc.dram_tensor(name, shape, dt, kind="ExternalInput").ap()

    xT = din("xT", [HID, S], bf16)
    # host-pretiled: row p, col block t = original rows 128t+p
    wq0 = din("wq0", [128, HID], bf16)
    wq1 = din("wq1", [128, HID], bf16)
    wkv = din("wkv", [128, HID], bf16)     # [wv | wk] columns pretiled
    wof = din("wof", [128, NK * HID], bf16)  # full wo, ktile-blocked
    cos2 = din("cos2", [128, S], f32)
    ss2 = din("ss2", [128, S], f32)
    ew_q = din("ew_q", [2, 128], f32r)
    ew_k = din("ew_k", [2, 128], f32r)
    e2 = din("e2", [2, 128], f32r)
    e2t = din("e2t", [128, 2], f32r)
    mask = din("mask", [128, 128], bf16)
    ident = din("ident", [64, 64], f32)

    out_hw = nc.dram_tensor("out", [256, S], f32,
                            kind="ExternalOutput").ap()

    with tile.TileContext(nc) as tc:
        with tc.tile_pool(name="consts", bufs=1) as cp, \
             tc.tile_pool(name="dram", bufs=1, space="DRAM") as dp, \
             tc.tile_pool(name="sbB", bufs=2) as sbB, \
             tc.tile_pool(name="sbC", bufs=4) as sbC, \
             tc.tile_pool(name="psO", bufs=2, space="PSUM") as psO:
            c_wq0 = cp.tile([128, HID], bf16, tag="w")
            c_wq1 = cp.tile([128, HID], bf16, tag="w2")
            c_wkv = cp.tile([128, HID], bf16, tag="w3")
            c_wof = cp.tile([128, NK * HID], bf16, tag="w4")
            c_cos = cp.tile([128, S], f32, tag="c1")
            c_ss = cp.tile([128, S], f32, tag="c2")
            c_ewq = cp.tile([2, 128], f32r, tag="c3")
            c_ewk = cp.tile([2, 128], f32r, tag="c4")
            c_e2 = cp.tile([2, 128], f32r, tag="c5")
            c_e2t = cp.tile([128, 2], f32r, tag="c5t")
            c_mask = cp.tile([128, 128], bf16, tag="c6")
            c_id = cp.tile([64, 64], f32, tag="c7")
            c_eps = cp.tile([2, 1], f32, tag="c8")

            # phase-A weights first (contiguous rows);
            # wq0 complete first so the first matmul unblocks early
            nc.sync.dma_start(c_wq0[:], wq0)
            nc.scalar.dma_start(c_wq1[:], wq1)
            nc.scalar.dma_start(c_wkv[:], wkv)
            nc.vector.memset(c_eps[:], RMS_EPS)
            nc.scalar.dma_start(c_e2t[:], e2t)
            nc.scalar.dma_start(c_ewq[:], ew_q)
            nc.scalar.dma_start(c_ewk[:], ew_k)
            nc.scalar.dma_start(c_id[:], ident)
            nc.scalar.dma_start(c_cos[:], cos2)
            nc.scalar.dma_start(c_ss[:], ss2)

            qr0 = cp.tile([128, S], bf16, tag="qr0")
            qr1 = cp.tile([128, S], bf16, tag="qr1")
            krd = cp.tile([128, S], bf16, tag="krd")
            v_aug = cp.tile([128, NKT * (HD + 1)], bf16, tag="vaug")

            a2a_in = [dp.tile([2048, 64], bf16, tag=f"ain{q}",
                              name=f"a2a_in{q}") for q in range(3)]
            a2a_out = [dp.tile([2048, 64], bf16, tag=f"aout{q}",
                               name=f"a2a_out{q}") for q in range(3)]
            a2a_in3 = [dp.tile([1024, 64], bf16, tag=f"ain3{h}",
                               name=f"a2a_in3{h}") for h in range(2)]
            a2a_out3 = [dp.tile([1024, 64], bf16, tag=f"aout3{h}",
                                name=f"a2a_out3{h}") for h in range(2)]

            specs = [
                ("kv", c_ewk, krd, True),
                ("q0", c_ewq, qr0, False),
                ("q1", c_ewq, qr1, False),
            ]
            qkv = {}   # per-half projection outputs, f32 [128, 1024]

            pqs = {}
            sqs = {}

            def qkv_copy(qh, j):
                key = ("q0", "q1", "kv")[j]
                dst = sbB.tile([128, 1024], f32, tag=f"qkv{j}",
                               bufs=1, name=f"qkv{qh}_{key}")
                nc.vector.tensor_copy(dst[:], pqs[qh][j][:])
                qkv[(qh, key)] = dst

            def stat_sq(qh, si):
                key = specs[si][0]
                sq = sbB.tile([128, 1024], f32r, tag="sq",
                              bufs=2, name=f"sq{qh}_{si}")
                nc.vector.tensor_mul(sq[:], qkv[(qh, key)][:],
                                     qkv[(qh, key)][:])
                sqs[(qh, si)] = sq

            def phase_a(qh, xp, psA, copies=True):
                """Projections for 1024-col half qh -> qkv f32 tiles."""
                hs = slice(1024 * qh, 1024 * qh + 1024)
                pq = [psA.tile([128, 1024], f32, tag="pa",
                               name=f"pa{qh}_{j}") for j in range(3)]
                for t in range(NK):
                    xt = xp.tile([128, 1024], bf16, tag="xt")
                    nc.sync.dma_start(xt[:],
                                      xT[128 * t:128 * (t + 1), hs])
                    st = (t == 0)
                    sp = (t == NK - 1)
                    tc_ = slice(128 * t, 128 * (t + 1))
                    for j, w in ((0, c_wq0), (1, c_wq1), (2, c_wkv)):
                        nc.tensor.matmul(pq[j][:, 0:512], w[:, tc_],
                                         xt[:, 0:512],
                                         start=st, stop=sp)
                        nc.tensor.matmul(pq[j][:, 512:1024], w[:, tc_],
                                         xt[:, 512:1024],
                                         start=st, stop=sp)
                pqs[qh] = pq
                if copies:
                    for j in range(3):
                        qkv_copy(qh, j)

            def phase_b_stats(qh):
                """RMS variance stats for half qh: partition sums via
                ones-block matmul, rstd = Exp(-0.5 Ln(var)) in place
                (batched per table: 2 ACT table loads)."""
                rstds = {}
                lnvs = {}
                for si in range(3):
                    if (qh, si) not in sqs:
                        stat_sq(qh, si)
                    sq = sqs[(qh, si)]
                    for u in range(2):
                        us = slice(512 * u, 512 * u + 512)
                        pss = psO.tile([2, 512], f32, tag="o",
                                       name=f"ss{qh}_{si}_{u}")
                        nc.tensor.matmul(pss[:], c_e2t[:], sq[:, us],
                                         start=True, stop=True)
                        lnv = sbB.tile([2, 512], f32r, tag="lnv",
                                       bufs=6, name=f"lnv{qh}{si}{u}")
                        nc.scalar.activation(lnv[:], pss[:], AF.Ln,
                                             scale=1.0 / HD,
                                             bias=c_eps[:])
                        lnvs[(si, u)] = lnv
                for si in range(3):
                    for u in range(2):
                        lnv = lnvs[(si, u)]
                        nc.scalar.activation(lnv[:], lnv[:],
                                             AF.Exp, scale=-0.5)
                        rstds[(si, u)] = lnv
                return rstds

            def phase_b_norm(qh, u, rstds):
                """Norm + RoPE for 512-col quarter (qh, u): DVE/DMA/
                TensorE only (no ScalarE), plus V transposes."""
                cs = slice(1024 * qh + 512 * u, 1024 * qh + 512 * u + 512)
                us = slice(512 * u, 512 * u + 512)
                for si, (key, ew, dst, is_kv) in enumerate(specs):
                    src = qkv[(qh, key)]
                    rows = slice(64, 128) if is_kv else slice(0, 128)
                    nrm = sbB.tile([128, 512], f32, tag="nrm",
                                   bufs=2, name=f"nrm{qh}{u}_{si}")
                    pb = psO.tile([128, 512], f32, tag="o",
                                  name=f"pb{qh}_{si}_{u}")
                    nc.tensor.matmul(pb[:], ew[:], rstds[(si, u)][:],
                                     start=True, stop=True)
                    nc.vector.tensor_mul(nrm[rows, :], src[rows, us],
                                         pb[rows, :])
                    # rope
                    sh = sbB.tile([128, 512], f32, tag="sh",
                                  bufs=2, name=f"sh{qh}{u}_{si}")
                    if is_kv:
                        nc.sync.dma_start(sh[64:96, :], nrm[96:128, :])
                        nc.sync.dma_start(sh[96:128, :], nrm[64:96, :])
                    else:
                        nc.sync.dma_start(sh[0:32, :], nrm[32:64, :])
                        nc.sync.dma_start(sh[32:64, :], nrm[0:32, :])
                        nc.sync.dma_start(sh[64:96, :], nrm[96:128, :])
                        nc.sync.dma_start(sh[96:128, :], nrm[64:96, :])
                    t2 = sbB.tile([128, 512], f32, tag="sq",
                                  bufs=2, name=f"t2{qh}{u}_{si}")
                    nc.vector.tensor_mul(t2[rows, :], sh[rows, :],
                                         c_ss[rows, cs])
                    t1 = sbB.tile([128, 512], f32, tag="sh",
                                  bufs=2, name=f"t1{qh}{u}_{si}")
                    nc.vector.tensor_mul(t1[rows, :], nrm[rows, :],
                                         c_cos[rows, cs])
                    nc.vector.tensor_add(dst[rows, cs], t1[rows, :],
                                         t2[rows, :])
                    if is_kv:
                        nc.sync.dma_start(dst[0:64, cs],
                                          dst[64:128, cs])
                        if qh == 0 and u == 0:
                            nc.gpsimd.memset(v_aug[:], 1.0)
                        for tt in range(8 * qh + 4 * u,
                                        8 * qh + 4 * u + 4):
                            lc = 128 * tt - 1024 * qh
                            ptr = psO.tile([128, 64], f32, tag="o",
                                           name=f"pt{qh}_{tt}")
                            nc.tensor.transpose(
                                ptr[:], src[0:64, lc:lc + 128], c_id[:])
                            nc.vector.tensor_copy(
                                v_aug[:,
                                      (HD + 1) * tt:(HD + 1) * tt + HD],
                                ptr[:])

            def cde_chunk(qc, psS, psPV, mid=None):
                """Scores + PV + normalize + A2A staging for q-chunk."""
                for hp, qr in ((0, qr0), (1, qr1)):
                    if hp == 1 and mid is not None:
                        mid()
                    ppv_a = psPV.tile([65, 512], f32, tag="pv")
                    ppv_b = psPV.tile([65, 512], f32, tag="pv")
                    ntile = 4 * qc + 4
                    for t in range(ntile):
                        r = t - 4 * qc
                        off = max(0, r) * 128
                        qlo = 512 * qc + off
                        qlen = 512 * (qc + 1) - qlo
                        kc = slice(128 * t, 128 * (t + 1))
                        vs = slice((HD + 1) * t, (HD + 1) * t + HD + 1)
                        st = (t == 0)
                        sp = (t == ntile - 1)
                        ps_s = psS.tile([128, 1024], f32, tag="s")
                        nc.tensor.matmul(
                            ps_s[:, 0:qlen], krd[0:64, kc],
                            qr[0:64, qlo:qlo + qlen],
                            start=True, stop=True)
                        nc.tensor.matmul(
                            ps_s[:, 512:512 + qlen], krd[64:128, kc],
                            qr[64:128, qlo:qlo + qlen],
                            start=True, stop=True)
                        pt = sbC.tile([128, 1024], bf16, tag="pt", bufs=3)
                        if r >= 0:
                            nc.scalar.activation(
                                pt[:, 0:qlen], ps_s[:, 0:qlen],
                                AF.Exp, scale=SCALING)
                            nc.scalar.activation(
                                pt[:, 512:512 + qlen],
                                ps_s[:, 512:512 + qlen],
                                AF.Exp, scale=SCALING)
                            nc.vector.tensor_mul(
                                pt[:, 0:128], pt[:, 0:128], c_mask[:])
                            nc.vector.tensor_mul(
                                pt[:, 512:640], pt[:, 512:640],
                                c_mask[:])
                        else:
                            nc.scalar.activation(
                                pt[:, 0:1024], ps_s[:, 0:1024],
                                AF.Exp, scale=SCALING)
                        nc.tensor.matmul(
                            ppv_a[:, off:512], v_aug[:, vs],
                            pt[:, 0:qlen], start=st, stop=sp)
                        nc.tensor.matmul(
                            ppv_b[:, off:512], v_aug[:, vs],
                            pt[:, 512:512 + qlen], start=st, stop=sp)
                    araw = sbC.tile([128, 512], f32, tag="araw",
                                    bufs=2, name=f"araw{qc}_{hp}")
                    lsb = sbC.tile([2, 512], f32, tag="lsb", bufs=1,
                                   name=f"lsb{qc}_{hp}")
                    for half, ppv in ((0, ppv_a), (1, ppv_b)):
                        stg = sbC.tile([65, 512], f32, tag="stg",
                                       bufs=2)
                        nc.vector.tensor_copy(stg[:], ppv[:])
                        nc.sync.dma_start(
                            araw[64 * half:64 * half + 64, :],
                            stg[0:64, :])
                        nc.sync.dma_start(
                            lsb[half:half + 1, :], stg[64:65, :])
                    # normalize: 1/l on DVE (keeps ScalarE exp table
                    # resident), broadcast via ones-block matmul
                    rl = sbC.tile([2, 512], f32, tag="lnl", bufs=1,
                                  name=f"rl{qc}_{hp}")
                    nc.vector.reciprocal_approx_fast(rl[:], lsb[:])
                    rl_r = sbC.tile([2, 512], f32r, tag="rlr", bufs=1,
                                    name=f"rlr{qc}_{hp}")
                    nc.vector.tensor_copy(rl_r[:], rl[:])
                    pb = psO.tile([128, 512], f32, tag="o")
                    nc.tensor.matmul(pb[:], c_e2[:], rl_r[:],
                                     start=True, stop=True)
                    abf = sbC.tile([128, 512], bf16, tag="abf",
                                   bufs=2, name=f"abf{qc}_{hp}")
                    nc.vector.tensor_mul(abf[:], araw[:], pb[:])
                    # stage bf16 attn into the AllToAll input:
                    # owner r gets seq cols 64r..64r+64 of this chunk.
                    # qc3 exchanges per-hp so hp0's collective overlaps
                    # hp1's attention (shortens the tail).
                    abf_r = abf[:].rearrange("p (r c) -> p r c",
                                             r=8, c=64)
                    if qc == 3:
                        nc.scalar.dma_start(
                            a2a_in3[hp][:].rearrange(
                                "(r p) c -> p r c", r=8, p=128),
                            abf_r)
                        nc.gpsimd.collective_compute(
                            "AllToAll",
                            mybir.AluOpType.bypass,
                            replica_groups=[list(range(N_CORES))],
                            ins=[a2a_in3[hp][:].opt()],
                            outs=[a2a_out3[hp][:].opt()],
                        )
                    else:
                        nc.scalar.dma_start(
                            a2a_in[qc][:].rearrange(
                                "(r h p) c -> h p r c",
                                r=8, h=2, p=128)[hp],
                            abf_r)
                if qc != 3:
                    nc.gpsimd.collective_compute(
                        "AllToAll",
                        mybir.AluOpType.bypass,
                        replica_groups=[list(range(N_CORES))],
                        ins=[a2a_in[qc][:].opt()],
                        outs=[a2a_out[qc][:].opt()],
                    )

            psSp = [None]
            lhs_t = [sbC.tile([128, NK * 128], bf16, tag="lhs",
                              bufs=2, name=f"lhsA{p}") for p in range(2)]

            def lhs_load(p, part):
                """part 0: chunk2p + (p1: hp0-even) cols; part 1: odd"""
                lhs = lhs_t[p]
                lhs2 = lhs[:].rearrange("p (t b c) -> b p t c",
                                        t=16, b=2, c=64)
                if part == 0:
                    nc.gpsimd.dma_start(
                        lhs2[0],
                        a2a_out[2 * p][:].rearrange(
                            "(t p) c -> p t c", t=16, p=128))
                    if p == 0:
                        nc.gpsimd.dma_start(
                            lhs2[1],
                            a2a_out[1][:].rearrange(
                                "(t p) c -> p t c", t=16, p=128))
                elif part == 2:
                    lhs4 = lhs[:].rearrange(
                        "p (j q b c) -> q b p j c",
                        j=8, q=2, b=2, c=64)
                    nc.gpsimd.dma_start(
                        lhs4[0][1],
                        a2a_out3[0][:].rearrange(
                            "(t p) c -> p t c", t=8, p=128))
                else:
                    lhs4 = lhs_t[1][:].rearrange(
                        "p (j q b c) -> q b p j c", j=8, q=2, b=2, c=64)
                    nc.gpsimd.dma_start(
                        lhs4[1][1],
                        a2a_out3[1][:].rearrange(
                            "(t p) c -> p t c", t=8, p=128))

            def oproj_rowpair(p):
                """o_proj for owner rows from q-chunks 2p, 2p+1."""
                lhs = lhs_t[p]
                if p == 1:
                    lhs_load(1, 1)
                ost = sbC.tile([128, S], f32, tag="ost", bufs=1,
                               name=f"ost{p}")

                def mm(po, n, t, st, sp):
                    nc.tensor.matmul(
                        po[:], lhs[:, 128 * t:128 * (t + 1)],
                        c_wof[:, HID * t + 512 * n:
                              HID * t + 512 * n + 512],
                        start=st, stop=sp)

                if p == 1:
                    # three waves: chunk2-only (needs just A2A_2),
                    # chunk3-even (hp0 exchange), chunk3-odd (hp1) —
                    # waves 1-2 fill the collective-wait gaps. Extra
                    # accumulators come from the now-idle psS pool.
                    def mmh(po, rows, n, t, cols, st, sp):
                        nc.tensor.matmul(
                            po[rows, :], lhs[:, cols],
                            c_wof[:, HID * t + 512 * n:
                                  HID * t + 512 * n + 512],
                            start=st, stop=sp)

                    pos = []
                    r2 = slice(0, 64)
                    r3 = slice(64, 128)
                    for n in range(4):
                        pool = psO if n < 2 else psSp[0]
                        po = pool.tile([128, 512], f32,
                                       tag="o" if n < 2 else "s",
                                       name=f"poe{n}")
                        pos.append(po)
                        for t in range(NK):   # wave 1: chunk2 cols
                            mmh(po, r2, n, t,
                                slice(128 * t, 128 * t + 64),
                                t == 0, t == NK - 1)
                    for n in range(4):        # wave 2: chunk3 even t
                        po = pos[n]
                        for i, t in enumerate(range(0, NK, 2)):
                            mmh(po, r3, n, t,
                                slice(128 * t + 64, 128 * t + 128),
                                i == 0, False)
                    for n in range(4):        # wave 3: chunk3 odd t
                        po = pos[n]
                        for i, t in enumerate(range(1, NK, 2)):
                            mmh(po, r3, n, t,
                                slice(128 * t + 64, 128 * t + 128),
                                False, i == NK // 2 - 1)
                        nc.vector.tensor_copy(
                            ost[:, 512 * n:512 * n + 512], po[:])
                        nc.sync.dma_start(
                            out_hw[128 * p:128 * (p + 1),
                                   512 * n:512 * n + 512],
                            ost[:, 512 * n:512 * n + 512])
                else:
                    for n in range(4):
                        po = psO.tile([128, 512], f32, tag="o")
                        for t in range(NK):
                            mm(po, n, t, t == 0, t == NK - 1)
                        nc.vector.tensor_copy(
                            ost[:, 512 * n:512 * n + 512], po[:])
                    nc.sync.dma_start(out_hw[128 * p:128 * (p + 1), :],
                                      ost[:])

            # ---- Projections: qh0 A+B, then qh1 A (hides qh0-B) ----
            with tc.tile_pool(name="xt", bufs=4) as xp, \
                 tc.tile_pool(name="psA", bufs=3, space="PSUM") as psA:
                phase_a(0, xp, psA)
                r0 = phase_b_stats(0)
                phase_b_norm(0, 0, r0)
                phase_b_norm(0, 1, r0)
                phase_a(1, xp, psA)

            # consts for CDE + full-wo prefetch (streams during qc0-2)
            nc.scalar.dma_start(c_e2[:], e2)
            nc.scalar.dma_start(c_mask[:], mask)
            for t in range(NK):
                ws = slice(HID * t, HID * (t + 1))
                nc.sync.dma_start(c_wof[:, ws], wof[:, ws])

            # ---- Attention + o_proj ----
            with tc.tile_pool(name="psS", bufs=2, space="PSUM") as psS, \
                 tc.tile_pool(name="psPV", bufs=2, space="PSUM") as psPV:
                psSp[0] = psS
                stat_sq(1, 0)            # spec si=0 -> kv
                cde_chunk(0, psS, psPV,
                          mid=lambda: stat_sq(1, 1))
                stat_sq(1, 2)            # spec si=2 -> q1
                r1 = phase_b_stats(1)
                phase_b_norm(1, 0, r1)   # DVE overlaps qc1 TensorE
                cde_chunk(1, psS, psPV)
                phase_b_norm(1, 1, r1)   # DVE overlaps qc2 TensorE
                cde_chunk(2, psS, psPV,
                          mid=lambda: lhs_load(0, 0))
                oproj_rowpair(0)
                lhs_load(1, 0)           # chunk2 cols: after A2A_2
                cde_chunk(3, psS, psPV,
                          mid=lambda: lhs_load(1, 2))
                oproj_rowpair(1)

    nc.compile()
    return nc


def _host_prep(hidden_states, position_ids, wq, wk, wv, wo, q_ln_w, k_ln_w):
    x = np.asarray(hidden_states, dtype=np.float32)[0]        # [S, HID]
    xT = np.ascontiguousarray(x.T).astype(BF16_NP)            # [HID, S]
    pos = np.asarray(position_ids)[0].astype(np.float32)      # [S]
    inv = 1.0 / (ROPE_THETA ** (np.arange(0, HD, 2, dtype=np.float32) / HD))
    ang = pos[:, None] * inv[None, :]                         # [S, 32]
    emb = np.concatenate([ang, ang], axis=1)                  # [S, 64]
    cosT = np.cos(emb).T.astype(np.float32)                   # [64, S]
    sinT = np.sin(emb).T.astype(np.float32)
    ss = sinT.copy()
    ss[0:32] = -sinT[0:32]
    cos2 = np.tile(cosT, (2, 1))
    ss2 = np.tile(ss, (2, 1))

    e2 = np.zeros((2, 128), dtype=np.float32)
    e2[0, 0:64] = 1.0
    e2[1, 64:128] = 1.0
    ew_q = np.zeros((2, 128), dtype=np.float32)
    ew_q[0, 0:64] = q_ln_w
    ew_q[1, 64:128] = q_ln_w
    ew_k = np.zeros((2, 128), dtype=np.float32)
    ew_k[1, 64:128] = k_ln_w
    msk = (np.arange(128)[:, None] <= np.arange(128)[None, :]) \
        .astype(BF16_NP)
    ident = np.eye(64, dtype=np.float32)

    wq_ = np.asarray(wq, dtype=np.float32)
    wk_ = np.asarray(wk, dtype=np.float32)
    wv_ = np.asarray(wv, dtype=np.float32)
    wo_ = np.asarray(wo, dtype=np.float32)

    def pretile(w):  # [HID, n] -> [128, NK*n] ktile-blocked
        n = w.shape[1]
        return np.ascontiguousarray(
            w.reshape(NK, 128, n).transpose(1, 0, 2).reshape(128, NK * n)
        ).astype(BF16_NP)

    wof = pretile(wo_)                                        # [128, 32768]

    in_maps = []
    for c in range(N_CORES):
        qcols = slice(256 * c, 256 * (c + 1))
        kvcols = slice(64 * c, 64 * (c + 1))
        wq_c = np.ascontiguousarray(wq_[:, qcols])
        wkv_c = np.concatenate([wv_[:, kvcols], wk_[:, kvcols]], axis=1)
        in_maps.append({
            "xT": xT,
            "wq0": pretile(wq_c[:, 0:128]),
            "wq1": pretile(wq_c[:, 128:256]),
            "wkv": pretile(wkv_c),
            "wof": wof,
            "cos2": cos2,
            "ss2": ss2,
            "ew_q": ew_q,
            "ew_k": ew_k,
            "e2": e2,
            "e2t": np.ascontiguousarray(e2.T),
            "mask": msk,
            "ident": ident,
        })
    return in_maps


def kernel(hidden_states, position_ids, wq, wk, wv, wo, q_ln_w, k_ln_w):
    global _NC_CACHE, LAST_RESULTS
    if _NC_CACHE is None:
        _NC_CACHE = _build()
    nc = _NC_CACHE
    in_maps = _host_prep(hidden_states, position_ids, wq, wk, wv, wo,
                         q_ln_w, k_ln_w)
    res = bass_utils.run_bass_kernel_spmd(
        nc, in_maps, core_ids=list(range(N_CORES)))
    LAST_RESULTS = res
    out = np.empty((S, HID), dtype=np.float32)
    for c in range(N_CORES):
        o_c = res.results[c]["out"]           # [256, 2048]
        for qc in range(NQC):
            out[512 * qc + 64 * c:512 * qc + 64 * c + 64, :] = \
                o_c[64 * qc:64 * qc + 64, :]
    return out.reshape(1, S, HID)


# revision 28
# speedup vs baseline: 1.1124x; 1.0051x over previous
"""GQA attention (B=1, S=2048, H=2048, 32 q-heads / 8 kv-heads, hd=64)
on 8 Trainium2 NeuronCores.

Sharding: tensor-parallel over heads. Core c owns q-heads 4c..4c+3 and
kv-head c. Each core AllToAlls its normalized attention output (bf16,
256 KB per q-chunk collective) so core r ends up with attnT
[2048 head-dims, seq] for its own interleaved 64-row sequence stripes,
then computes o_proj locally against the FULL wo (bf16, SBUF-resident,
prefetched during attention). Core r owns output rows
{512*qc + 64*r + [0,64)}.

All matmul inputs are bf16 (1 cyc/row on the PE at any width, half the
DMA/SBUF traffic); PSUM accumulation stays fp32 and the RMSNorm / RoPE
/ softmax-stat math stays fp32/f32r on DVE/ScalarE.

Schedule (program order = engine queue order per engine):
  qh0-A proj -> qh0-B norm/rope -> qh1-A proj   (qh1-A matmuls hide
    qh0-B's DVE chain)
  CDE qc0 -> A2A0 -> qc1 -> A2A1 -> qh1-B (DVE overlaps qc0/1 TensorE)
  qc2 -> o_proj rows{qc0,qc1} -> qc3 -> A2A3 -> o_proj rows{qc2,qc3}
PSUM banks: psA(6) + psO(2) during projections; psS(4) + psPV(2) +
psO(2) during attention (psA closed first).
"""
import sys

sys.path.insert(0, "/opt/trn_rl_repo")

import numpy as np  # noqa: E402
import concourse.bacc as bacc  # noqa: E402
import concourse.mybir as mybir  # noqa: E402
import concourse.tile as tile  # noqa: E402
from concourse import bass_utils  # noqa: E402

f32 = mybir.dt.float32
f32r = mybir.dt.float32r
bf16 = mybir.dt.bfloat16
AF = mybir.ActivationFunctionType
BF16_NP = mybir.dt.np(bf16)

N_CORES = 8
S = 2048
HID = 2048
HD = 64
ROPE_THETA = 10000.0
RMS_EPS = 1e-6
SCALING = HD ** -0.5              # 0.125
NK = HID // 128                   # 16 contraction tiles
NQC = S // 512                    # 4 q chunks
NKT = S // 128                    # 16 kpos tiles

_NC_CACHE = None
LAST_RESULTS = None


def _build():
    nc = bacc.Bacc("TRN2", target_bir_lowering=False, debug=False,
                   num_devices=N_CORES)

    def din(name, shape, dt):
        return nc.dram_tensor(name, shape, dt, kind="ExternalInput").ap()

    xT = din("xT", [HID, S], bf16)
    # host-pretiled: row p, col block t = original rows 128t+p
    wq0 = din("wq0", [128, HID], bf16)
    wq1 = din("wq1", [128, HID], bf16)
    wkv = din("wkv", [128, HID], bf16)     # [wv | wk] columns pretiled
    wof = din("wof", [128, NK * HID], bf16)  # full wo, ktile-blocked
    cos2 = din("cos2", [128, S], f32)
    ss2 = din("ss2", [128, S], f32)
    ew_q = din("ew_q", [2, 128], f32r)
    ew_k = din("ew_k", [2, 128], f32r)
    e2 = din("e2", [2, 128], f32r)
    e2t = din("e2t", [128, 2], f32r)
    mask = din("mask", [128, 128], bf16)
    ident = din("ident", [64, 64], f32)

    out_hw = nc.dram_tensor("out", [256, S], f32,
                            kind="ExternalOutput").ap()

    with tile.TileContext(nc) as tc:
        with tc.tile_pool(name="consts", bufs=1) as cp, \
             tc.tile_pool(name="dram", bufs=1, space="DRAM") as dp, \
             tc.tile_pool(name="sbB", bufs=2) as sbB, \
             tc.tile_pool(name="sbC", bufs=4) as sbC, \
             tc.tile_pool(name="psO", bufs=2, space="PSUM") as psO:
            c_wq0 = cp.tile([128, HID], bf16, tag="w")
            c_wq1 = cp.tile([128, HID], bf16, tag="w2")
            c_wkv = cp.tile([128, HID], bf16, tag="w3")
            c_wof = cp.tile([128, NK * HID], bf16, tag="w4")
            c_cos = cp.tile([128, S], f32, tag="c1")
            c_ss = cp.tile([128, S], f32, tag="c2")
            c_ewq = cp.tile([2, 128], f32r, tag="c3")
            c_ewk = cp.tile([2, 128], f32r, tag="c4")
            c_e2 = cp.tile([2, 128], f32r, tag="c5")
            c_e2t = cp.tile([128, 2], f32r, tag="c5t")
            c_mask = cp.tile([128, 128], bf16, tag="c6")
            c_id = cp.tile([64, 64], f32, tag="c7")
            c_eps = cp.tile([2, 1], f32, tag="c8")

            # phase-A weights first (contiguous rows);
            # wq0 complete first so the first matmul unblocks early
            nc.sync.dma_start(c_wq0[:], wq0)
            nc.scalar.dma_start(c_wq1[:], wq1)
            nc.scalar.dma_start(c_wkv[:], wkv)
            nc.vector.memset(c_eps[:], RMS_EPS)
            nc.scalar.dma_start(c_e2t[:], e2t)
            nc.scalar.dma_start(c_ewq[:], ew_q)
            nc.scalar.dma_start(c_ewk[:], ew_k)
            nc.scalar.dma_start(c_id[:], ident)

            qr0 = cp.tile([128, S], bf16, tag="qr0")
            qr1 = cp.tile([128, S], bf16, tag="qr1")
            krd = cp.tile([128, S], bf16, tag="krd")
            v_aug = cp.tile([128, NKT * (HD + 1)], bf16, tag="vaug")

            a2a_in = [dp.tile([2048, 64], bf16, tag=f"ain{q}",
                              name=f"a2a_in{q}") for q in range(3)]
            a2a_out = [dp.tile([2048, 64], bf16, tag=f"aout{q}",
                               name=f"a2a_out{q}") for q in range(3)]
            a2a_in3 = [dp.tile([1024, 64], bf16, tag=f"ain3{h}",
                               name=f"a2a_in3{h}") for h in range(2)]
            a2a_out3 = [dp.tile([1024, 64], bf16, tag=f"aout3{h}",
                                name=f"a2a_out3{h}") for h in range(2)]

            specs = [
                ("kv", c_ewk, krd, True),
                ("q0", c_ewq, qr0, False),
                ("q1", c_ewq, qr1, False),
            ]
            qkv = {}   # per-half projection outputs, f32 [128, 1024]

            pqs = {}
            sqs = {}

            def qkv_copy(qh, j):
                key = ("q0", "q1", "kv")[j]
                dst = sbB.tile([128, 1024], f32, tag=f"qkv{j}",
                               bufs=1, name=f"qkv{qh}_{key}")
                nc.vector.tensor_copy(dst[:], pqs[qh][j][:])
                qkv[(qh, key)] = dst

            def stat_sq(qh, si):
                key = specs[si][0]
                sq = sbB.tile([128, 1024], f32r, tag="sq",
                              bufs=2, name=f"sq{qh}_{si}")
                nc.vector.tensor_mul(sq[:], qkv[(qh, key)][:],
                                     qkv[(qh, key)][:])
                sqs[(qh, si)] = sq

            def phase_a(qh, xp, psA, copies=True):
                """Projections for 1024-col half qh -> qkv f32 tiles."""
                hs = slice(1024 * qh, 1024 * qh + 1024)
                pq = [psA.tile([128, 1024], f32, tag="pa",
                               name=f"pa{qh}_{j}") for j in range(3)]
                for t in range(NK):
                    xt = xp.tile([128, 1024], bf16, tag="xt")
                    nc.sync.dma_start(xt[:],
                                      xT[128 * t:128 * (t + 1), hs])
                    if qh == 0 and t == 12:
                        # delay the 2MB rope tables until the startup
                        # xt burst is done (WAW guard on a corner elem)
                        for cdst, csrc in ((c_cos, cos2), (c_ss, ss2)):
                            nc.vector.tensor_copy(cdst[0:1, 0:2],
                                                  xt[0:1, 0:2])
                            nc.scalar.dma_start(cdst[:], csrc)
                    st = (t == 0)
                    sp = (t == NK - 1)
                    tc_ = slice(128 * t, 128 * (t + 1))
                    for j, w in ((0, c_wq0), (1, c_wq1), (2, c_wkv)):
                        nc.tensor.matmul(pq[j][:, 0:512], w[:, tc_],
                                         xt[:, 0:512],
                                         start=st, stop=sp)
                        nc.tensor.matmul(pq[j][:, 512:1024], w[:, tc_],
                                         xt[:, 512:1024],
                                         start=st, stop=sp)
                pqs[qh] = pq
                if copies:
                    for j in range(3):
                        qkv_copy(qh, j)

            def phase_b_stats(qh):
                """RMS variance stats for half qh: partition sums via
                ones-block matmul, rstd = Exp(-0.5 Ln(var)) in place
                (batched per table: 2 ACT table loads)."""
                rstds = {}
                lnvs = {}
                for si in range(3):
                    if (qh, si) not in sqs:
                        stat_sq(qh, si)
                    sq = sqs[(qh, si)]
                    for u in range(2):
                        us = slice(512 * u, 512 * u + 512)
                        pss = psO.tile([2, 512], f32, tag="o",
                                       name=f"ss{qh}_{si}_{u}")
                        nc.tensor.matmul(pss[:], c_e2t[:], sq[:, us],
                                         start=True, stop=True)
                        lnv = sbB.tile([2, 512], f32r, tag="lnv",
                                       bufs=6, name=f"lnv{qh}{si}{u}")
                        nc.scalar.activation(lnv[:], pss[:], AF.Ln,
                                             scale=1.0 / HD,
                                             bias=c_eps[:])
                        lnvs[(si, u)] = lnv
                for si in range(3):
                    for u in range(2):
                        lnv = lnvs[(si, u)]
                        nc.scalar.activation(lnv[:], lnv[:],
                                             AF.Exp, scale=-0.5)
                        rstds[(si, u)] = lnv
                return rstds

            def phase_b_norm(qh, u, rstds):
                """Norm + RoPE for 512-col quarter (qh, u): DVE/DMA/
                TensorE only (no ScalarE), plus V transposes."""
                cs = slice(1024 * qh + 512 * u, 1024 * qh + 512 * u + 512)
                us = slice(512 * u, 512 * u + 512)
                for si, (key, ew, dst, is_kv) in enumerate(specs):
                    src = qkv[(qh, key)]
                    rows = slice(64, 128) if is_kv else slice(0, 128)
                    nrm = sbB.tile([128, 512], f32, tag="nrm",
                                   bufs=2, name=f"nrm{qh}{u}_{si}")
                    pb = psO.tile([128, 512], f32, tag="o",
                                  name=f"pb{qh}_{si}_{u}")
                    nc.tensor.matmul(pb[:], ew[:], rstds[(si, u)][:],
                                     start=True, stop=True)
                    nc.vector.tensor_mul(nrm[rows, :], src[rows, us],
                                         pb[rows, :])
                    # rope
                    sh = sbB.tile([128, 512], f32, tag="sh",
                                  bufs=2, name=f"sh{qh}{u}_{si}")
                    if is_kv:
                        nc.sync.dma_start(sh[64:96, :], nrm[96:128, :])
                        nc.sync.dma_start(sh[96:128, :], nrm[64:96, :])
                    else:
                        nc.sync.dma_start(sh[0:32, :], nrm[32:64, :])
                        nc.sync.dma_start(sh[32:64, :], nrm[0:32, :])
                        nc.sync.dma_start(sh[64:96, :], nrm[96:128, :])
                        nc.sync.dma_start(sh[96:128, :], nrm[64:96, :])
                    t2 = sbB.tile([128, 512], f32, tag="sq",
                                  bufs=2, name=f"t2{qh}{u}_{si}")
                    nc.vector.tensor_mul(t2[rows, :], sh[rows, :],
                                         c_ss[rows, cs])
                    t1 = sbB.tile([128, 512], f32, tag="sh",
                                  bufs=2, name=f"t1{qh}{u}_{si}")
                    nc.vector.tensor_mul(t1[rows, :], nrm[rows, :],
                                         c_cos[rows, cs])
                    nc.vector.tensor_add(dst[rows, cs], t1[rows, :],
                                         t2[rows, :])
                    if is_kv:
                        nc.sync.dma_start(dst[0:64, cs],
                                          dst[64:128, cs])
                        if qh == 0 and u == 0:
                            nc.gpsimd.memset(v_aug[:], 1.0)
                        for tt in range(8 * qh + 4 * u,
                                        8 * qh + 4 * u + 4):
                            lc = 128 * tt - 1024 * qh
                            ptr = psO.tile([128, 64], f32, tag="o",
                                           name=f"pt{qh}_{tt}")
                            nc.tensor.transpose(
                                ptr[:], src[0:64, lc:lc + 128], c_id[:])
                            nc.vector.tensor_copy(
                                v_aug[:,
                                      (HD + 1) * tt:(HD + 1) * tt + HD],
                                ptr[:])

            def cde_chunk(qc, psS, psPV, mid=None):
                """Scores + PV + normalize + A2A staging for q-chunk."""
                for hp, qr in ((0, qr0), (1, qr1)):
                    if hp == 1 and mid is not None:
                        mid()
                    ppv_a = psPV.tile([65, 512], f32, tag="pv")
                    ppv_b = psPV.tile([65, 512], f32, tag="pv")
                    ntile = 4 * qc + 4
                    for t in range(ntile):
                        r = t - 4 * qc
                        off = max(0, r) * 128
                        qlo = 512 * qc + off
                        qlen = 512 * (qc + 1) - qlo
                        kc = slice(128 * t, 128 * (t + 1))
                        vs = slice((HD + 1) * t, (HD + 1) * t + HD + 1)
                        st = (t == 0)
                        sp = (t == ntile - 1)
                        ps_s = psS.tile([128, 1024], f32, tag="s")
                        nc.tensor.matmul(
                            ps_s[:, 0:qlen], krd[0:64, kc],
                            qr[0:64, qlo:qlo + qlen],
                            start=True, stop=True)
                        nc.tensor.matmul(
                            ps_s[:, 512:512 + qlen], krd[64:128, kc],
                            qr[64:128, qlo:qlo + qlen],
                            start=True, stop=True)
                        pt = sbC.tile([128, 1024], bf16, tag="pt", bufs=3)
                        if r >= 0:
                            nc.scalar.activation(
                                pt[:, 0:qlen], ps_s[:, 0:qlen],
                                AF.Exp, scale=SCALING)
                            nc.scalar.activation(
                                pt[:, 512:512 + qlen],
                                ps_s[:, 512:512 + qlen],
                                AF.Exp, scale=SCALING)
                            nc.vector.tensor_mul(
                                pt[:, 0:128], pt[:, 0:128], c_mask[:])
                            nc.vector.tensor_mul(
                                pt[:, 512:640], pt[:, 512:640],
                                c_mask[:])
                        else:
                            nc.scalar.activation(
                                pt[:, 0:1024], ps_s[:, 0:1024],
                                AF.Exp, scale=SCALING)
                        nc.tensor.matmul(
                            ppv_a[:, off:512], v_aug[:, vs],
                            pt[:, 0:qlen], start=st, stop=sp)
                        nc.tensor.matmul(
                            ppv_b[:, off:512], v_aug[:, vs],
                            pt[:, 512:512 + qlen], start=st, stop=sp)
                    araw = sbC.tile([128, 512], f32, tag="araw",
                                    bufs=2, name=f"araw{qc}_{hp}")
                    lsb = sbC.tile([2, 512], f32, tag="lsb", bufs=1,
                                   name=f"lsb{qc}_{hp}")
                    for half, ppv in ((0, ppv_a), (1, ppv_b)):
                        stg = sbC.tile([65, 512], f32, tag="stg",
                                       bufs=2)
                        nc.vector.tensor_copy(stg[:], ppv[:])
                        nc.sync.dma_start(
                            araw[64 * half:64 * half + 64, :],
                            stg[0:64, :])
                        nc.sync.dma_start(
                            lsb[half:half + 1, :], stg[64:65, :])
                    # normalize: 1/l on DVE (keeps ScalarE exp table
                    # resident), broadcast via ones-block matmul
                    rl = sbC.tile([2, 512], f32, tag="lnl", bufs=1,
                                  name=f"rl{qc}_{hp}")
                    nc.vector.reciprocal_approx_fast(rl[:], lsb[:])
                    rl_r = sbC.tile([2, 512], f32r, tag="rlr", bufs=1,
                                    name=f"rlr{qc}_{hp}")
                    nc.vector.tensor_copy(rl_r[:], rl[:])
                    pb = psO.tile([128, 512], f32, tag="o")
                    nc.tensor.matmul(pb[:], c_e2[:], rl_r[:],
                                     start=True, stop=True)
                    abf = sbC.tile([128, 512], bf16, tag="abf",
                                   bufs=2, name=f"abf{qc}_{hp}")
                    nc.vector.tensor_mul(abf[:], araw[:], pb[:])
                    # stage bf16 attn into the AllToAll input:
                    # owner r gets seq cols 64r..64r+64 of this chunk.
                    # qc3 exchanges per-hp so hp0's collective overlaps
                    # hp1's attention (shortens the tail).
                    abf_r = abf[:].rearrange("p (r c) -> p r c",
                                             r=8, c=64)
                    if qc == 3:
                        nc.scalar.dma_start(
                            a2a_in3[hp][:].rearrange(
                                "(r p) c -> p r c", r=8, p=128),
                            abf_r)
                        nc.gpsimd.collective_compute(
                            "AllToAll",
                            mybir.AluOpType.bypass,
                            replica_groups=[list(range(N_CORES))],
                            ins=[a2a_in3[hp][:].opt()],
                            outs=[a2a_out3[hp][:].opt()],
                        )
                    else:
                        nc.scalar.dma_start(
                            a2a_in[qc][:].rearrange(
                                "(r h p) c -> h p r c",
                                r=8, h=2, p=128)[hp],
                            abf_r)
                if qc != 3:
                    nc.gpsimd.collective_compute(
                        "AllToAll",
                        mybir.AluOpType.bypass,
                        replica_groups=[list(range(N_CORES))],
                        ins=[a2a_in[qc][:].opt()],
                        outs=[a2a_out[qc][:].opt()],
                    )

            psSp = [None]
            lhs_t = [sbC.tile([128, NK * 128], bf16, tag="lhs",
                              bufs=2, name=f"lhsA{p}") for p in range(2)]

            def lhs_load(p, part):
                """part 0: chunk2p + (p1: hp0-even) cols; part 1: odd"""
                lhs = lhs_t[p]
                lhs2 = lhs[:].rearrange("p (t b c) -> b p t c",
                                        t=16, b=2, c=64)
                if part == 0:
                    nc.gpsimd.dma_start(
                        lhs2[0],
                        a2a_out[2 * p][:].rearrange(
                            "(t p) c -> p t c", t=16, p=128))
                    if p == 0:
                        nc.gpsimd.dma_start(
                            lhs2[1],
                            a2a_out[1][:].rearrange(
                                "(t p) c -> p t c", t=16, p=128))
                elif part == 2:
                    lhs4 = lhs[:].rearrange(
                        "p (j q b c) -> q b p j c",
                        j=8, q=2, b=2, c=64)
                    nc.gpsimd.dma_start(
                        lhs4[0][1],
                        a2a_out3[0][:].rearrange(
                            "(t p) c -> p t c", t=8, p=128))
                else:
                    lhs4 = lhs_t[1][:].rearrange(
                        "p (j q b c) -> q b p j c", j=8, q=2, b=2, c=64)
                    nc.gpsimd.dma_start(
                        lhs4[1][1],
                        a2a_out3[1][:].rearrange(
                            "(t p) c -> p t c", t=8, p=128))

            def oproj_rowpair(p):
                """o_proj for owner rows from q-chunks 2p, 2p+1."""
                lhs = lhs_t[p]
                if p == 1:
                    lhs_load(1, 1)
                ost = sbC.tile([128, S], f32, tag="ost", bufs=1,
                               name=f"ost{p}")

                def mm(po, n, t, st, sp):
                    nc.tensor.matmul(
                        po[:], lhs[:, 128 * t:128 * (t + 1)],
                        c_wof[:, HID * t + 512 * n:
                              HID * t + 512 * n + 512],
                        start=st, stop=sp)

                if p == 1:
                    # three waves: chunk2-only (needs just A2A_2),
                    # chunk3-even (hp0 exchange), chunk3-odd (hp1) —
                    # waves 1-2 fill the collective-wait gaps. Extra
                    # accumulators come from the now-idle psS pool.
                    def mmh(po, rows, n, t, cols, st, sp):
                        nc.tensor.matmul(
                            po[rows, :], lhs[:, cols],
                            c_wof[:, HID * t + 512 * n:
                                  HID * t + 512 * n + 512],
                            start=st, stop=sp)

                    pos = []
                    r2 = slice(0, 64)
                    r3 = slice(64, 128)
                    for n in range(4):
                        pool = psO if n < 2 else psSp[0]
                        po = pool.tile([128, 512], f32,
                                       tag="o" if n < 2 else "s",
                                       name=f"poe{n}")
                        pos.append(po)
                        for t in range(NK):   # wave 1: chunk2 cols
                            mmh(po, r2, n, t,
                                slice(128 * t, 128 * t + 64),
                                t == 0, t == NK - 1)
                    for n in range(4):        # wave 2: chunk3 even t
                        po = pos[n]
                        for i, t in enumerate(range(0, NK, 2)):
                            mmh(po, r3, n, t,
                                slice(128 * t + 64, 128 * t + 128),
                                i == 0, False)
                    for n in range(4):        # wave 3: chunk3 odd t
                        po = pos[n]
                        for i, t in enumerate(range(1, NK, 2)):
                            mmh(po, r3, n, t,
                                slice(128 * t + 64, 128 * t + 128),
                                False, i == NK // 2 - 1)
                        nc.vector.tensor_copy(
                            ost[:, 512 * n:512 * n + 512], po[:])
                        nc.sync.dma_start(
                            out_hw[128 * p:128 * (p + 1),
                                   512 * n:512 * n + 512],
                            ost[:, 512 * n:512 * n + 512])
                else:
                    for n in range(4):
                        po = psO.tile([128, 512], f32, tag="o")
                        for t in range(NK):
                            mm(po, n, t, t == 0, t == NK - 1)
                        nc.vector.tensor_copy(
                            ost[:, 512 * n:512 * n + 512], po[:])
                    nc.sync.dma_start(out_hw[128 * p:128 * (p + 1), :],
                                      ost[:])

            # ---- Projections: qh0 A+B, then qh1 A (hides qh0-B) ----
            with tc.tile_pool(name="xt", bufs=4) as xp, \
                 tc.tile_pool(name="psA", bufs=3, space="PSUM") as psA:
                phase_a(0, xp, psA)
                r0 = phase_b_stats(0)
                phase_b_norm(0, 0, r0)
                phase_b_norm(0, 1, r0)
                phase_a(1, xp, psA)

            # consts for CDE + full-wo prefetch (streams during qc0-2)
            nc.scalar.dma_start(c_e2[:], e2)
            nc.scalar.dma_start(c_mask[:], mask)
            for t in range(NK):
                ws = slice(HID * t, HID * (t + 1))
                nc.sync.dma_start(c_wof[:, ws], wof[:, ws])

            # ---- Attention + o_proj ----
            with tc.tile_pool(name="psS", bufs=2, space="PSUM") as psS, \
                 tc.tile_pool(name="psPV", bufs=2, space="PSUM") as psPV:
                psSp[0] = psS
                stat_sq(1, 0)            # spec si=0 -> kv
                cde_chunk(0, psS, psPV,
                          mid=lambda: stat_sq(1, 1))
                stat_sq(1, 2)            # spec si=2 -> q1
                r1 = phase_b_stats(1)
                phase_b_norm(1, 0, r1)   # DVE overlaps qc1 TensorE
                cde_chunk(1, psS, psPV)
                phase_b_norm(1, 1, r1)   # DVE overlaps qc2 TensorE
                cde_chunk(2, psS, psPV,
                          mid=lambda: lhs_load(0, 0))
                oproj_rowpair(0)
                lhs_load(1, 0)           # chunk2 cols: after A2A_2
                cde_chunk(3, psS, psPV,
                          mid=lambda: lhs_load(1, 2))
                oproj_rowpair(1)

    nc.compile()
    return nc


def _host_prep(hidden_states, position_ids, wq, wk, wv, wo, q_ln_w, k_ln_w):
    x = np.asarray(hidden_states, dtype=np.float32)[0]        # [S, HID]
    xT = np.ascontiguousarray(x.T).astype(BF16_NP)            # [HID, S]
    pos = np.asarray(position_ids)[0].astype(np.float32)      # [S]
    inv = 1.0 / (ROPE_THETA ** (np.arange(0, HD, 2, dtype=np.float32) / HD))
    ang = pos[:, None] * inv[None, :]                         # [S, 32]
    emb = np.concatenate([ang, ang], axis=1)                  # [S, 64]
    cosT = np.cos(emb).T.astype(np.float32)                   # [64, S]
    sinT = np.sin(emb).T.astype(np.float32)
    ss = sinT.copy()
    ss[0:32] = -sinT[0:32]
    cos2 = np.tile(cosT, (2, 1))
    ss2 = np.tile(ss, (2, 1))

    e2 = np.zeros((2, 128), dtype=np.float32)
    e2[0, 0:64] = 1.0
    e2[1, 64:128] = 1.0
    ew_q = np.zeros((2, 128), dtype=np.float32)
    ew_q[0, 0:64] = q_ln_w
    ew_q[1, 64:128] = q_ln_w
    ew_k = np.zeros((2, 128), dtype=np.float32)
    ew_k[1, 64:128] = k_ln_w
    msk = (np.arange(128)[:, None] <= np.arange(128)[None, :]) \
        .astype(BF16_NP)
    ident = np.eye(64, dtype=np.float32)

    wq_ = np.asarray(wq, dtype=np.float32)
    wk_ = np.asarray(wk, dtype=np.float32)
    wv_ = np.asarray(wv, dtype=np.float32)
    wo_ = np.asarray(wo, dtype=np.float32)

    def pretile(w):  # [HID, n] -> [128, NK*n] ktile-blocked
        n = w.shape[1]
        return np.ascontiguousarray(
            w.reshape(NK, 128, n).transpose(1, 0, 2).reshape(128, NK * n)
        ).astype(BF16_NP)

    wof = pretile(wo_)                                        # [128, 32768]

    in_maps = []
    for c in range(N_CORES):
        qcols = slice(256 * c, 256 * (c + 1))
        kvcols = slice(64 * c, 64 * (c + 1))
        wq_c = np.ascontiguousarray(wq_[:, qcols])
        wkv_c = np.concatenate([wv_[:, kvcols], wk_[:, kvcols]], axis=1)
        in_maps.append({
            "xT": xT,
            "wq0": pretile(wq_c[:, 0:128]),
            "wq1": pretile(wq_c[:, 128:256]),
            "wkv": pretile(wkv_c),
            "wof": wof,
            "cos2": cos2,
            "ss2": ss2,
            "ew_q": ew_q,
            "ew_k": ew_k,
            "e2": e2,
            "e2t": np.ascontiguousarray(e2.T),
            "mask": msk,
            "ident": ident,
        })
    return in_maps


def kernel(hidden_states, position_ids, wq, wk, wv, wo, q_ln_w, k_ln_w):
    global _NC_CACHE, LAST_RESULTS
    if _NC_CACHE is None:
        _NC_CACHE = _build()
    nc = _NC_CACHE
    in_maps = _host_prep(hidden_states, position_ids, wq, wk, wv, wo,
                         q_ln_w, k_ln_w)
    res = bass_utils.run_bass_kernel_spmd(
        nc, in_maps, core_ids=list(range(N_CORES)))
    LAST_RESULTS = res
    out = np.empty((S, HID), dtype=np.float32)
    for c in range(N_CORES):
        o_c = res.results[c]["out"]           # [256, 2048]
        for qc in range(NQC):
            out[512 * qc + 64 * c:512 * qc + 64 * c + 64, :] = \
                o_c[64 * qc:64 * qc + 64, :]
    return out.reshape(1, S, HID)
